# revision 27
# baseline (speedup 1.0000x reference)
"""Trainium2 Bass kernel for nn_BatchSplitFF (expert-choice MoE routing + FFN).

Strategy (data-parallel over batch, 1 batch per NeuronCore, 8 cores):
  - fp32 logits on PE in [es, tok] orientation, N=512 token chunks (routing
    must match the fp32 reference argmax; bf16 logits would flip decisions).
  - routing: group max (V) -> one-hot iseq (GpSimd) -> token ids (GpSimd/V)
    exported for the host-side combine.
  - dispatch ON-CHIP via PE one-hot matmuls; dispatch of half 0 is fused
    into the logits phase, software-pipelined one chunk behind routing so
    PE never waits on the routing V/Gp chain and selT evacuation spreads
    over the whole logits phase instead of rate-limiting its own phase.
  - expert FFN in bf16 on PE with array tiling (up: es-pair column tiles,
    down: es-pair row tiles), relu/bias on ScalarE.
  - un-permute: y rows written DENSELY (one row per (es, g)) with contiguous
    DMAs; host does the final scatter-add combine in fp32.
Host side only reshapes/casts/transposes inputs and combines the output.

Inherited measured-and-REJECTED variants from earlier sessions (do not
retry): DMA gather dispatch (~150us/queue descriptor overhead); float32r
logits (routing flips); quarter-granular dispatch N=128 (+30us PE);
per-pair up/down interleave (mode-switch drains); fp8 weights/selT/xl
(error budget); on-chip combine via perm matmul (y transpose >= write
savings); logits [ch|cl] term-merge (cross-partition fold impossible on
DVE/ACT - lanes are partition-aligned).
"""

import numpy as np
import ml_dtypes

import concourse.bass as bass
import concourse.mybir as mybir
import concourse.tile as tile
from concourse import bacc
from concourse.bass_utils import run_bass_kernel_spmd

bf16 = ml_dtypes.bfloat16
f32 = mybir.dt.float32
f32r = mybir.dt.float32r
bfl = mybir.dt.bfloat16
i32 = mybir.dt.int32

DM, NE, ES, ESZ = 1024, 16, 4, 64
NES = NE * ES            # 64 (e,s) expert pairs
SEQ = 2048
G = SEQ // NE            # 128 groups per core
KT = DM // 128           # 8 contraction tiles
NCORES = 8
NTT = SEQ // 128         # 16 token tiles (8 groups each)
GPT = 8                  # groups per token tile
EHALF = NES // 2         # 32 es per dispatch half
NPAIR = NES // 2         # 32 es pairs
PPH = NPAIR // 2         # 16 pairs per half

_CACHE = {}


def _build_program():
    nc = bacc.Bacc("TRN2", target_bir_lowering=False, debug=False,
                   enable_asserts=False, num_devices=1)

    # pre-tiled logits chunks: [chunk, p, hi/lo, k, tok]
    xthl = nc.dram_tensor("xthl", [4, 128, 2, KT, 512], bfl,
                          kind="ExternalInput").ap()
    # natural-layout bf16 x wrapped [(r p) d -> p r d]
    xbf = nc.dram_tensor("xbf", [128, NTT, DM], bfl, kind="ExternalInput").ap()
    # controller hi/lo pre-tiled [p, k, e] so the load is 128 contiguous rows
    c2h = nc.dram_tensor("c2h", [128, KT, NES], bfl, kind="ExternalInput").ap()
    c2l = nc.dram_tensor("c2l", [128, KT, NES], bfl, kind="ExternalInput").ap()
    # pre-tiled f1 octets: [octet, p, k, 8*ESZ]
    f1w = nc.dram_tensor("f1w", [8, 128, KT, 8 * ESZ], bfl,
                         kind="ExternalInput").ap()
    # f2 stacked in es pairs, pre-tiled per 4-pair chunk: [ac, 2*64 f, 4, DM]
    f2w = nc.dram_tensor("f2w", [8, 128, 4, DM], bfl, kind="ExternalInput").ap()
    bias2 = nc.dram_tensor("bias2", [128, NPAIR], f32, kind="ExternalInput").ap()
    tie512 = nc.dram_tensor("tie512", [NES, 512], f32, kind="ExternalInput").ap()
    tokid = nc.dram_tensor("tokid", [NES, SEQ], f32, kind="ExternalInput").ap()
    ident = nc.dram_tensor("ident", [NES, NES], f32, kind="ExternalInput").ap()
    glmask = nc.dram_tensor("glmask", [128, GPT], bfl, kind="ExternalInput").ap()
    # [g, es, d] so each pair write is one contiguous 4KB run per partition
    stage3 = nc.dram_tensor("stage3", [G, NES, DM], bfl,
                            kind="ExternalOutput").ap()
    tids = nc.dram_tensor("tids", [NES, G], i32, kind="ExternalOutput").ap()

    with tile.TileContext(nc) as tc:
        with (
            tc.tile_pool(name="consts", bufs=1) as consts,
            tc.tile_pool(name="route", bufs=1) as route,
        ):
            # ---- first constants into SBUF (pre-tiled: fast descriptors) ----
            ch_sb = consts.tile([128, KT, NES], bfl)
            nc.sync.dma_start(out=ch_sb[:], in_=c2h)
            cl_sb = consts.tile([128, KT, NES], bfl)
            nc.sync.dma_start(out=cl_sb[:], in_=c2l)
            tie_sb = consts.tile([NES, 512], f32)
            nc.sync.dma_start(out=tie_sb[:], in_=tie512)
            bias_sb = consts.tile([128, NPAIR], f32)
            tokid_sb = consts.tile([NES, SEQ], f32)
            ident_sb = consts.tile([NES, NES], f32)
            glmask_sb = consts.tile([128, GPT], bfl)
            x_sb = consts.tile([128, NTT, DM], bfl)

            # PE warmup on ch_sb right after its (fast) load: releases the
            # HAM clock gate so real matmuls run at 2.4 GHz
            with tc.tile_pool(name="warm", bufs=1, space="PSUM") as warm:
                wps = warm.tile([NES, 512], f32, space="PSUM")
                wrhs = ch_sb.rearrange("p k e -> p (k e)")
                for _ in range(8):
                    nc.tensor.matmul(wps[:], ch_sb[:, 0, :], wrhs,
                                     start=True, stop=True)

            perm_all = route.tile([128, NTT, NES, GPT], bfl)

            with (
                tc.tile_pool(name="selp", bufs=1) as selp,
                tc.tile_pool(name="wp1", bufs=4) as wp1,
            ):
                sel_tiles = {}
                h_tiles = {}
                f1_tiles = {}
                f2_tiles = {}
                ecnt = [0]

                def ealt():
                    ecnt[0] += 1
                    return ecnt[0] % 2 == 0

                def emit_disp_chunk(psD, half, i):
                    # i in 0..31: (tt, kq) chunk of 4 k-tiles, N=256
                    tt, kq = divmod(i, 2)
                    e0 = half * EHALF
                    if i == 0:
                        sel_tiles[half] = selp.tile(
                            [128, KT, NTT, EHALF, GPT], bfl, tag="selT",
                            name=f"selT_{half}")
                    selT = sel_tiles[half]
                    psum_s = psD.tile([128, 4, 256], f32, space="PSUM",
                                      tag=f"d{i % 2}")
                    for kk in range(4):
                        k = kq * 4 + kk
                        nc.tensor.matmul(
                            psum_s[:, kk, :],
                            x_sb[:, tt, k * 128:(k + 1) * 128],
                            perm_all[:, tt, e0:e0 + EHALF, :],
                            start=True, stop=True)
                    dst = selT[:, kq * 4:(kq + 1) * 4, tt, :, :]
                    sc = psum_s.rearrange("p k (e g) -> p k e g", e=EHALF)
                    if ealt():
                        nc.vector.tensor_copy(out=dst, in_=sc)
                    else:
                        nc.scalar.copy(out=dst, in_=sc)

                def emit_up_pair(psH, half, j):
                    # j in 0..15: es pair within half, col-tiled 128x64
                    selT = sel_tiles[half]
                    if j == 0:
                        h_tiles[half] = dcp.tile([128, PPH, G], bfl, tag="h",
                                                 name=f"h_{half}")
                    h_all = h_tiles[half]
                    a8, jj = divmod(j, 4)
                    ag = 4 * half + a8             # global octet
                    if ag not in f1_tiles:
                        f1_sb = wp1.tile([128, KT, 8 * ESZ], bfl, tag="f1",
                                         name=f"f1_{ag}")
                        nc.sync.dma_start(out=f1_sb[:], in_=f1w[ag])
                        f1_tiles[ag] = f1_sb
                    f1_sb = f1_tiles[ag]
                    jg = half * PPH + j            # global pair
                    psum_h = psH.tile([128, G], f32, space="PSUM")
                    el = jj * 2
                    for k in range(KT):
                        nc.tensor.matmul(
                            psum_h[0:64, :],
                            f1_sb[:, k, el * ESZ:(el + 1) * ESZ],
                            selT[:, k, :, a8 * 8 + el, :],
                            start=(k == 0), stop=(k == KT - 1),
                            tile_position=(0, 0))
                        nc.tensor.matmul(
                            psum_h[64:128, :],
                            f1_sb[:, k, (el + 1) * ESZ:(el + 2) * ESZ],
                            selT[:, k, :, a8 * 8 + el + 1, :],
                            start=(k == 0), stop=(k == KT - 1),
                            tile_position=(0, 64))
                    nc.scalar.activation(
                        out=h_all[:, j, :], in_=psum_h[:],
                        func=mybir.ActivationFunctionType.Relu,
                        bias=bias_sb[:, jg:jg + 1], scale=1.0)

                def emit_down_pair(psY, half, j, single=False):
                    # j in 0..15: es pair, row-tiled 64x128
                    h_all = h_tiles[half]
                    jg = half * PPH + j
                    ac, jj = divmod(jg, 4)
                    # prefetch the f2 chunk two 4-pair blocks ahead so the
                    # 1MB load never gates a down pair
                    ac_next = ac + 2
                    if j % 4 == 0 and ac_next < 8 and ac_next not in f2_tiles:
                        prefetch_f2(ac_next)
                    if ac not in f2_tiles:
                        prefetch_f2(ac)
                    f2_sb = f2_tiles[ac]
                    if single:
                        # one 4-bank psum tile; 2 big evacs (V and S run
                        # concurrently), halving sem traffic per pair
                        ps4 = psY.tile([128, 4, 512], f32, space="PSUM",
                                       tag="y", name=f"py_{jg}")
                        ps = [ps4[:, m, :] for m in range(4)]
                    else:
                        ps = [psY.tile([128, 512], f32, space="PSUM",
                                       tag=f"y{m}", name=f"py_{jg}_{m}")
                              for m in range(4)]
                    for n in range(2):
                        nc.tensor.matmul(
                            ps[n],
                            h_all[0:64, j, :],
                            f2_sb[0:64, jj, n * 512:(n + 1) * 512],
                            start=True, stop=True, tile_position=(0, 0))
                        nc.tensor.matmul(
                            ps[2 + n],
                            h_all[64:128, j, :],
                            f2_sb[64:128, jj, n * 512:(n + 1) * 512],
                            start=True, stop=True, tile_position=(64, 0))
                    y_sb = yp.tile([128, 2, DM], bfl)
                    if single:
                        nc.vector.tensor_copy(
                            out=y_sb[:, 0, :].rearrange("p (n c) -> p n c",
                                                        n=2),
                            in_=ps4[:, 0:2, :])
                        nc.scalar.copy(
                            out=y_sb[:, 1, :].rearrange("p (n c) -> p n c",
                                                        n=2),
                            in_=ps4[:, 2:4, :])
                    else:
                        for m in range(4):
                            dst = y_sb[:, m // 2,
                                       (m % 2) * 512:(m % 2 + 1) * 512]
                            if ealt():
                                nc.vector.tensor_copy(out=dst, in_=ps[m])
                            else:
                                nc.scalar.copy(out=dst, in_=ps[m])
                    # dense write: rows (g, es) for es = 2*jg, 2*jg+1
                    nc.sync.dma_start(
                        out=stage3[:, 2 * jg:2 * jg + 2, :],
                        in_=y_sb[:])

                def prefetch_f1(ag):
                    f1_sb = wp1.tile([128, KT, 8 * ESZ], bfl, tag="f1",
                                     name=f"f1_{ag}")
                    nc.sync.dma_start(out=f1_sb[:], in_=f1w[ag])
                    f1_tiles[ag] = f1_sb

                def prefetch_f2(ac):
                    f2_sb = wp2.tile([128, 4, DM], bfl, tag="f2",
                                     name=f"f2_{ac}")
                    nc.sync.dma_start(out=f2_sb[:], in_=f2w[ac])
                    f2_tiles[ac] = f2_sb

                # ---- fused phase: logits + routing + dispatch half 0 ----
                # dispatch for chunk c-1 is emitted inside chunk c so PE has
                # ready work while the routing V/Gp chain for chunk c runs.
                # psD spans the fused phase AND dispatch half 1 (down0 block).
                psD_cm = tc.tile_pool(name="psD", bufs=1, space="PSUM")
                psD = psD_cm.__enter__()
                with (
                    tc.tile_pool(name="xtp", bufs=2) as xtp,
                    tc.tile_pool(name="rt", bufs=1) as rt,
                    tc.tile_pool(name="psB", bufs=2, space="PSUM") as psB,
                    tc.tile_pool(name="psC", bufs=2, space="PSUM") as psC,
                ):
                    logits_sb = rt.tile([NES, SEQ], f32)
                    iseq = rt.tile([NES, SEQ], f32)
                    # tsel overwrites logits in place (logits chunk is dead
                    # once gmax+iseq for that chunk are computed)
                    tsel = logits_sb
                    gmax = rt.tile([NES, G], f32)
                    tid_f = rt.tile([NES, G], f32)
                    xt_tiles = []
                    # DMA queue order: xt0 hi, xt0 lo, small consts, xb0,
                    # xt1 hi/lo, xb1, ...
                    for tc4 in range(SEQ // 512):
                        xt_t = xtp.tile([128, 2, KT, 512], bfl, tag="xt",
                                        name=f"xt_{tc4}")
                        nc.sync.dma_start(out=xt_t[:, 0], in_=xthl[tc4, :, 0])
                        nc.sync.dma_start(out=xt_t[:, 1], in_=xthl[tc4, :, 1])
                        xt_tiles.append(xt_t)
                        if tc4 == 0:
                            nc.sync.dma_start(out=ident_sb[:], in_=ident)
                            nc.sync.dma_start(out=glmask_sb[:], in_=glmask)
                            nc.sync.dma_start(out=tokid_sb[:], in_=tokid)
                        nc.sync.dma_start(
                            out=x_sb[:, 4 * tc4:4 * tc4 + 4, :],
                            in_=xbf[:, 4 * tc4:4 * tc4 + 4, :])
                        if tc4 == 3:
                            nc.sync.dma_start(out=bias_sb[:], in_=bias2)
                            prefetch_f1(0)
                            prefetch_f1(1)

                    def emit_logits(xt_t, psum_l, c0, c1, first, last):
                        # fp32-split logits, hi terms first so the PE can
                        # start before the lo half of the chunk lands
                        for k in range(KT):
                            nc.tensor.matmul(psum_l[:, c0:c1],
                                             ch_sb[:, k, :],
                                             xt_t[:, 0, k, c0:c1],
                                             start=(first and k == 0),
                                             stop=False)
                            nc.tensor.matmul(psum_l[:, c0:c1],
                                             cl_sb[:, k, :],
                                             xt_t[:, 0, k, c0:c1],
                                             start=False, stop=False)
                        for k in range(KT):
                            nc.tensor.matmul(psum_l[:, c0:c1],
                                             ch_sb[:, k, :],
                                             xt_t[:, 1, k, c0:c1],
                                             start=False,
                                             stop=(last and k == KT - 1))

                    def emit_routing(psum_l, t0, t1, g0, g1, c0, c1):
                        # tiebreak add during PSUM->SBUF, then group argmax
                        nc.vector.tensor_tensor(
                            out=logits_sb[:, t0:t1],
                            in0=psum_l[:, c0:c1], in1=tie_sb[:, c0:c1],
                            op=mybir.AluOpType.add)
                        ng = g1 - g0
                        lg = logits_sb[:, t0:t1].rearrange(
                            "e (g t) -> e g t", t=NE)
                        nc.vector.tensor_reduce(
                            out=gmax[:, g0:g1], in_=lg,
                            axis=mybir.AxisListType.X, op=mybir.AluOpType.max)
                        nc.vector.tensor_tensor(
                            out=iseq[:, t0:t1].rearrange(
                                "e (g t) -> e g t", t=NE),
                            in0=lg,
                            in1=gmax[:, g0:g1].unsqueeze(2).to_broadcast(
                                [NES, ng, NE]),
                            op=mybir.AluOpType.is_equal)
                        nc.gpsimd.tensor_tensor(
                            out=tsel[:, t0:t1], in0=iseq[:, t0:t1],
                            in1=tokid_sb[:, t0:t1], op=mybir.AluOpType.mult)
                        nc.vector.tensor_reduce(
                            out=tid_f[:, g0:g1],
                            in_=tsel[:, t0:t1].rearrange(
                                "e (g t) -> e g t", t=NE),
                            axis=mybir.AxisListType.X, op=mybir.AluOpType.max)

                    def emit_perm(tt):
                        # PE transpose + fused mask-broadcast mult on V
                        psum_t = psC.tile([128, NES], f32, space="PSUM")
                        nc.tensor.transpose(
                            out=psum_t[:],
                            in_=iseq[:, tt * 128:(tt + 1) * 128],
                            identity=ident_sb[:])
                        nc.vector.tensor_tensor(
                            out=perm_all[:, tt, :, :],
                            in0=psum_t.unsqueeze(2).to_broadcast(
                                [128, NES, GPT]),
                            in1=glmask_sb.unsqueeze(1).to_broadcast(
                                [128, NES, GPT]),
                            op=mybir.AluOpType.mult)

                    # chunk 0: fine-grained sub-chunk pipeline to minimize
                    # the cold-start routing-chain stall
                    xt_t = xt_tiles[0]
                    psum_l0 = psB.tile([NES, 512], f32, space="PSUM",
                                       tag="l", name="psl_0")
                    emit_logits(xt_t, psum_l0, 0, 256, True, True)
                    emit_routing(psum_l0, 0, 256, 0, 16, 0, 256)
                    emit_logits(xt_t, psum_l0, 256, 512, True, True)
                    emit_routing(psum_l0, 256, 512, 16, 32, 256, 512)
                    emit_perm(0)
                    emit_perm(1)
                    for i in range(0, 4):
                        emit_disp_chunk(psD, 0, i)
                    emit_perm(2)
                    emit_perm(3)

                    for tc4 in range(1, SEQ // 512):
                        xt_t = xt_tiles[tc4]
                        t0, t1 = tc4 * 512, (tc4 + 1) * 512
                        g0, g1 = tc4 * 32, (tc4 + 1) * 32
                        psum_l = psB.tile([NES, 512], f32, space="PSUM",
                                          tag="l", name=f"psl_{tc4}")
                        emit_logits(xt_t, psum_l, 0, 512, True, True)
                        emit_routing(psum_l, t0, t1, g0, g1, 0, 512)
                        # dispatch lagging one chunk behind routing (chunk 0
                        # already emitted its first half inline)
                        start = 4 if tc4 == 1 else 8 * (tc4 - 1)
                        for i in range(start, 8 * tc4):
                            emit_disp_chunk(psD, 0, i)
                        for tt in range(4 * tc4, 4 * tc4 + 4):
                            emit_perm(tt)
                    # trailing dispatch chunks
                    for i in range(24, 32):
                        emit_disp_chunk(psD, 0, i)
                    # export routing table for the host-side combine
                    tid_i32 = rt.tile([NES, G], i32)
                    nc.vector.tensor_copy(out=tid_i32[:], in_=tid_f[:])
                    nc.sync.dma_start(out=tids, in_=tid_i32[:])

                # ---- FFN phase ----
                with (
                    tc.tile_pool(name="dcp", bufs=1) as dcp,
                    tc.tile_pool(name="wp2", bufs=4) as wp2,
                    tc.tile_pool(name="yp", bufs=3) as yp,
                ):
                    prefetch_f1(2)
                    prefetch_f1(3)
                    for ac in range(3):
                        prefetch_f2(ac)
                    with tc.tile_pool(name="psH", bufs=2,
                                      space="PSUM") as psH:
                        for j in range(PPH):
                            emit_up_pair(psH, 0, j)
                            # f1 octets for half 1 load during up0/down0
                            if j % 4 == 3:
                                prefetch_f1(4 + j // 4)
                    with tc.tile_pool(name="psY", bufs=1,
                                      space="PSUM") as psY:
                        for j in range(PPH):
                            emit_down_pair(psY, 0, j)
                            for i in range(2 * j, 2 * j + 2):
                                emit_disp_chunk(psD, 1, i)
                    psD_cm.__exit__(None, None, None)
                    with tc.tile_pool(name="psH", bufs=4,
                                      space="PSUM") as psH:
                        for j in range(PPH):
                            emit_up_pair(psH, 1, j)
                    with tc.tile_pool(name="psY", bufs=2,
                                      space="PSUM") as psY:
                        for j in range(PPH):
                            emit_down_pair(psY, 1, j, single=True)

    nc.compile()
    return nc


def _host_prep(x, controller, f1, f2, bias):
    """Returns (shared_map, per_core_maps)."""
    x = np.asarray(x, dtype=np.float32)
    c2 = np.ascontiguousarray(np.asarray(controller, np.float32).reshape(DM, NES))
    c2h = c2.astype(bf16)
    c2l = (c2 - c2h.astype(np.float32)).astype(bf16)
    # pre-tile [dm, e] -> [p, k, e]
    c2h = np.ascontiguousarray(c2h.reshape(KT, 128, NES).transpose(1, 0, 2))
    c2l = np.ascontiguousarray(c2l.reshape(KT, 128, NES).transpose(1, 0, 2))
    f1m = np.asarray(f1, np.float32).reshape(DM, NES * ESZ).astype(bf16)
    # [octet, p, k, 512]: f1w[ag, p, k, q] = f1m[k*128+p, ag*512+q]
    f1w = np.ascontiguousarray(
        f1m.reshape(KT, 128, 8, 512).transpose(2, 1, 0, 3))
    # f2 stacked in es pairs: [(pair-parity f), pair, DM]
    f2p = np.asarray(f2, np.float32).reshape(NPAIR, 2, ESZ, DM)
    f2w = f2p.transpose(1, 2, 0, 3).reshape(128, NPAIR, DM)
    # pre-tile per 4-pair chunk: [ac, 128, 4, DM]
    f2w = np.ascontiguousarray(
        f2w.reshape(128, 8, 4, DM).transpose(1, 0, 2, 3)).astype(bf16)
    b2 = np.asarray(bias, np.float32).reshape(NPAIR, 2, ESZ)
    bias2 = np.ascontiguousarray(b2.transpose(1, 2, 0).reshape(128, NPAIR))
    tie = np.linspace(0.0, 1e-6, NE, dtype=np.float32)
    tie512 = np.broadcast_to(np.tile(tie, 512 // NE), (NES, 512)).copy()
    tokid = np.broadcast_to(np.arange(SEQ, dtype=np.float32), (NES, SEQ)).copy()
    ident = np.eye(NES, dtype=np.float32)
    gl = (np.arange(128) // NE)[:, None] == np.arange(GPT)[None, :]
    glmask = np.ascontiguousarray(gl.astype(bf16))
    shared = dict(c2h=c2h, c2l=c2l, f1w=f1w, f2w=f2w, bias2=bias2,
                  tie512=tie512, tokid=tokid, ident=ident, glmask=glmask)
    per_core = []
    for b in range(NCORES):
        xb = x[b]
        xT = np.ascontiguousarray(xb.T)
        xTh = xT.astype(bf16)
        xTl = (xT - xTh.astype(np.float32)).astype(bf16)
        # [chunk, p, hl, k, tok]: xthl[c, p, h, k, t] = xThl[k*128+p, c*512+t]
        xthl = np.stack([xTh, xTl], axis=1)          # [DM, 2, SEQ]
        xthl = np.ascontiguousarray(
            xthl.reshape(KT, 128, 2, 4, 512).transpose(3, 1, 2, 0, 4))
        per_core.append(dict(
            xthl=xthl,
            xbf=np.ascontiguousarray(
                xb.astype(bf16).reshape(NTT, 128, DM).transpose(1, 0, 2)),
        ))
    return shared, per_core


def _run(inputs, trace=False, tmpdir=None, trace_cores=None):
    if "nc" not in _CACHE:
        _CACHE["nc"] = _build_program()
    nc = _CACHE["nc"]
    shared, per_core = _host_prep(
        inputs["x"], inputs["controller"], inputs["f1"], inputs["f2"],
        inputs["bias"])
    in_maps = [dict(shared, **pc) for pc in per_core]
    res = run_bass_kernel_spmd(
        nc, in_maps, core_ids=list(range(NCORES)), trace=trace, tmpdir=tmpdir,
        trace_cores=trace_cores)
    out = np.zeros((NCORES, SEQ, DM), dtype=np.float32)
    for b in range(NCORES):
        # stage3[g, es, :] holds y for (es, group g); tids[es, g] is the token
        st = np.asarray(res.results[b]["stage3"]).astype(np.float32)
        tid = np.asarray(res.results[b]["tids"]).reshape(NES, G)
        np.add.at(out[b], tid.T.reshape(-1), st.reshape(G * NES, DM))
    return out, res


def kernel(**inputs) -> np.ndarray:
    out, _ = _run(inputs)
    return out


# revision 29
# speedup vs baseline: 1.0608x; 1.0608x over previous
"""Trainium2 Bass kernel for nn_BatchSplitFF (expert-choice MoE routing + FFN).

Strategy (data-parallel over batch, 1 batch per NeuronCore, 8 cores):
  - fp32 logits on PE in [es, tok] orientation, N=512 token chunks (routing
    must match the fp32 reference argmax; bf16 logits would flip decisions).
  - routing: group max (V) -> one-hot iseq (GpSimd) -> token ids (GpSimd/V)
    exported for the host-side combine.
  - dispatch ON-CHIP via PE one-hot matmuls; dispatch of half 0 is fused
    into the logits phase, software-pipelined one chunk behind routing so
    PE never waits on the routing V/Gp chain and selT evacuation spreads
    over the whole logits phase instead of rate-limiting its own phase.
  - expert FFN in bf16 on PE with array tiling (up: es-pair column tiles,
    down: es-pair row tiles), relu/bias on ScalarE.
  - un-permute: y rows written DENSELY (one row per (es, g)) with contiguous
    DMAs; host does the final scatter-add combine in fp32.
Host side only reshapes/casts/transposes inputs and combines the output.

Inherited measured-and-REJECTED variants from earlier sessions (do not
retry): DMA gather dispatch (~150us/queue descriptor overhead); float32r
logits (routing flips); quarter-granular dispatch N=128 (+30us PE);
per-pair up/down interleave (mode-switch drains); fp8 weights/selT/xl
(error budget); on-chip combine via perm matmul (y transpose >= write
savings); logits [ch|cl] term-merge (cross-partition fold impossible on
DVE/ACT - lanes are partition-aligned).
"""

import numpy as np
import ml_dtypes

import concourse.bass as bass
import concourse.mybir as mybir
import concourse.tile as tile
from concourse import bacc
from concourse.bass_utils import run_bass_kernel_spmd

bf16 = ml_dtypes.bfloat16
f32 = mybir.dt.float32
f32r = mybir.dt.float32r
bfl = mybir.dt.bfloat16
i32 = mybir.dt.int32

DM, NE, ES, ESZ = 1024, 16, 4, 64
NES = NE * ES            # 64 (e,s) expert pairs
SEQ = 2048
G = SEQ // NE            # 128 groups per core
KT = DM // 128           # 8 contraction tiles
NCORES = 8
NTT = SEQ // 128         # 16 token tiles (8 groups each)
GPT = 8                  # groups per token tile
EHALF = NES // 2         # 32 es per dispatch half
NPAIR = NES // 2         # 32 es pairs
PPH = NPAIR // 2         # 16 pairs per half

_CACHE = {}


def _build_program():
    nc = bacc.Bacc("TRN2", target_bir_lowering=False, debug=False,
                   enable_asserts=False, num_devices=1)

    # pre-tiled logits chunks: [chunk, p, hi/lo, k, tok]
    xthl = nc.dram_tensor("xthl", [4, 128, 2, KT, 512], bfl,
                          kind="ExternalInput").ap()
    # natural-layout bf16 x wrapped [(r p) d -> p r d]
    xbf = nc.dram_tensor("xbf", [128, NTT, DM], bfl, kind="ExternalInput").ap()
    # controller hi/lo pre-tiled [p, k, e] so the load is 128 contiguous rows
    c2h = nc.dram_tensor("c2h", [128, KT, NES], bfl, kind="ExternalInput").ap()
    c2l = nc.dram_tensor("c2l", [128, KT, NES], bfl, kind="ExternalInput").ap()
    # pre-tiled f1 octets: [octet, p, k, 8*ESZ]
    f1w = nc.dram_tensor("f1w", [8, 128, KT, 8 * ESZ], bfl,
                         kind="ExternalInput").ap()
    # f2 stacked in es pairs, pre-tiled per 4-pair chunk: [ac, 2*64 f, 4, DM]
    f2w = nc.dram_tensor("f2w", [8, 128, 4, DM], bfl, kind="ExternalInput").ap()
    bias2 = nc.dram_tensor("bias2", [128, NPAIR], f32, kind="ExternalInput").ap()
    tie512 = nc.dram_tensor("tie512", [NES, 512], f32, kind="ExternalInput").ap()
    tokid = nc.dram_tensor("tokid", [NES, SEQ], f32, kind="ExternalInput").ap()
    ident = nc.dram_tensor("ident", [NES, NES], f32, kind="ExternalInput").ap()
    glmask = nc.dram_tensor("glmask", [128, GPT], bfl, kind="ExternalInput").ap()
    # [g, es, d] so each pair write is one contiguous 4KB run per partition
    stage3 = nc.dram_tensor("stage3", [G, NES, DM], bfl,
                            kind="ExternalOutput").ap()
    tids = nc.dram_tensor("tids", [NES, G], i32, kind="ExternalOutput").ap()

    with tile.TileContext(nc) as tc:
        with (
            tc.tile_pool(name="consts", bufs=1) as consts,
            tc.tile_pool(name="route", bufs=1) as route,
        ):
            # ---- first constants into SBUF (pre-tiled: fast descriptors) ----
            ch_sb = consts.tile([128, KT, NES], bfl)
            nc.sync.dma_start(out=ch_sb[:], in_=c2h)
            cl_sb = consts.tile([128, KT, NES], bfl)
            nc.sync.dma_start(out=cl_sb[:], in_=c2l)
            tie_sb = consts.tile([NES, 512], f32)
            nc.sync.dma_start(out=tie_sb[:], in_=tie512)
            bias_sb = consts.tile([128, NPAIR], f32)
            tokid_sb = consts.tile([NES, SEQ], f32)
            ident_sb = consts.tile([NES, NES], f32)
            glmask_sb = consts.tile([128, GPT], bfl)
            x_sb = consts.tile([128, NTT, DM], bfl)

            # PE warmup on ch_sb right after its (fast) load: releases the
            # HAM clock gate so real matmuls run at 2.4 GHz
            with tc.tile_pool(name="warm", bufs=1, space="PSUM") as warm:
                wps = warm.tile([NES, 512], f32, space="PSUM")
                wrhs = ch_sb.rearrange("p k e -> p (k e)")
                for _ in range(8):
                    nc.tensor.matmul(wps[:], ch_sb[:, 0, :], wrhs,
                                     start=True, stop=True)

            perm_all = route.tile([128, NTT, NES, GPT], bfl)

            with (
                tc.tile_pool(name="selp", bufs=1) as selp,
                tc.tile_pool(name="wp1", bufs=4) as wp1,
            ):
                sel_tiles = {}
                h_tiles = {}
                f1_tiles = {}
                f2_tiles = {}
                ecnt = [0]

                def ealt():
                    ecnt[0] += 1
                    return ecnt[0] % 2 == 0

                def emit_disp_chunk(psD, half, i):
                    # i in 0..31: (tt, kq) chunk of 4 k-tiles, N=256
                    tt, kq = divmod(i, 2)
                    e0 = half * EHALF
                    if i == 0:
                        sel_tiles[half] = selp.tile(
                            [128, KT, NTT, EHALF, GPT], bfl, tag="selT",
                            name=f"selT_{half}")
                    selT = sel_tiles[half]
                    psum_s = psD.tile([128, 4, 256], f32, space="PSUM",
                                      tag=f"d{i % 2}")
                    for kk in range(4):
                        k = kq * 4 + kk
                        nc.tensor.matmul(
                            psum_s[:, kk, :],
                            x_sb[:, tt, k * 128:(k + 1) * 128],
                            perm_all[:, tt, e0:e0 + EHALF, :],
                            start=True, stop=True)
                    dst = selT[:, kq * 4:(kq + 1) * 4, tt, :, :]
                    sc = psum_s.rearrange("p k (e g) -> p k e g", e=EHALF)
                    if ealt():
                        nc.vector.tensor_copy(out=dst, in_=sc)
                    else:
                        nc.scalar.copy(out=dst, in_=sc)

                def emit_up_pair(psH, half, j):
                    # j in 0..15: es pair within half, col-tiled 128x64
                    selT = sel_tiles[half]
                    if j == 0:
                        h_tiles[half] = dcp.tile([128, PPH, G], bfl, tag="h",
                                                 name=f"h_{half}")
                    h_all = h_tiles[half]
                    a8, jj = divmod(j, 4)
                    ag = 4 * half + a8             # global octet
                    if ag not in f1_tiles:
                        f1_sb = wp1.tile([128, KT, 8 * ESZ], bfl, tag="f1",
                                         name=f"f1_{ag}")
                        nc.sync.dma_start(out=f1_sb[:], in_=f1w[ag])
                        f1_tiles[ag] = f1_sb
                    f1_sb = f1_tiles[ag]
                    jg = half * PPH + j            # global pair
                    psum_h = psH.tile([128, G], f32, space="PSUM")
                    el = jj * 2
                    for k in range(KT):
                        nc.tensor.matmul(
                            psum_h[0:64, :],
                            f1_sb[:, k, el * ESZ:(el + 1) * ESZ],
                            selT[:, k, :, a8 * 8 + el, :],
                            start=(k == 0), stop=(k == KT - 1),
                            tile_position=(0, 0))
                        nc.tensor.matmul(
                            psum_h[64:128, :],
                            f1_sb[:, k, (el + 1) * ESZ:(el + 2) * ESZ],
                            selT[:, k, :, a8 * 8 + el + 1, :],
                            start=(k == 0), stop=(k == KT - 1),
                            tile_position=(0, 64))
                    nc.scalar.activation(
                        out=h_all[:, j, :], in_=psum_h[:],
                        func=mybir.ActivationFunctionType.Relu,
                        bias=bias_sb[:, jg:jg + 1], scale=1.0)

                def emit_down_pair(psY, half, j, single=False):
                    # j in 0..15: es pair, row-tiled 64x128
                    h_all = h_tiles[half]
                    jg = half * PPH + j
                    ac, jj = divmod(jg, 4)
                    # prefetch the f2 chunk two 4-pair blocks ahead so the
                    # 1MB load never gates a down pair
                    ac_next = ac + 2
                    if j % 4 == 0 and ac_next < 8 and ac_next not in f2_tiles:
                        prefetch_f2(ac_next)
                    if ac not in f2_tiles:
                        prefetch_f2(ac)
                    f2_sb = f2_tiles[ac]
                    if single:
                        # one 4-bank psum tile; 2 big evacs (V and S run
                        # concurrently), halving sem traffic per pair
                        ps4 = psY.tile([128, 4, 512], f32, space="PSUM",
                                       tag="y", name=f"py_{jg}")
                        ps = [ps4[:, m, :] for m in range(4)]
                    else:
                        ps = [psY.tile([128, 512], f32, space="PSUM",
                                       tag=f"y{m}", name=f"py_{jg}_{m}")
                              for m in range(4)]
                    for n in range(2):
                        nc.tensor.matmul(
                            ps[n],
                            h_all[0:64, j, :],
                            f2_sb[0:64, jj, n * 512:(n + 1) * 512],
                            start=True, stop=True, tile_position=(0, 0))
                        nc.tensor.matmul(
                            ps[2 + n],
                            h_all[64:128, j, :],
                            f2_sb[64:128, jj, n * 512:(n + 1) * 512],
                            start=True, stop=True, tile_position=(64, 0))
                    y_sb = yp.tile([128, 2, DM], bfl)
                    if single:
                        nc.vector.tensor_copy(
                            out=y_sb[:, 0, :].rearrange("p (n c) -> p n c",
                                                        n=2),
                            in_=ps4[:, 0:2, :])
                        nc.scalar.copy(
                            out=y_sb[:, 1, :].rearrange("p (n c) -> p n c",
                                                        n=2),
                            in_=ps4[:, 2:4, :])
                    else:
                        for m in range(4):
                            dst = y_sb[:, m // 2,
                                       (m % 2) * 512:(m % 2 + 1) * 512]
                            if ealt():
                                nc.vector.tensor_copy(out=dst, in_=ps[m])
                            else:
                                nc.scalar.copy(out=dst, in_=ps[m])
                    # dense write: rows (g, es) for es = 2*jg, 2*jg+1
                    nc.sync.dma_start(
                        out=stage3[:, 2 * jg:2 * jg + 2, :],
                        in_=y_sb[:])

                def prefetch_f1(ag):
                    f1_sb = wp1.tile([128, KT, 8 * ESZ], bfl, tag="f1",
                                     name=f"f1_{ag}")
                    nc.sync.dma_start(out=f1_sb[:], in_=f1w[ag])
                    f1_tiles[ag] = f1_sb

                def prefetch_f2(ac):
                    f2_sb = wp2.tile([128, 4, DM], bfl, tag="f2",
                                     name=f"f2_{ac}")
                    nc.sync.dma_start(out=f2_sb[:], in_=f2w[ac])
                    f2_tiles[ac] = f2_sb

                # ---- fused phase: logits + routing + dispatch half 0 ----
                # dispatch for chunk c-1 is emitted inside chunk c so PE has
                # ready work while the routing V/Gp chain for chunk c runs.
                # psD spans the fused phase AND dispatch half 1 (down0 block).
                psD_cm = tc.tile_pool(name="psD", bufs=1, space="PSUM")
                psD = psD_cm.__enter__()
                with (
                    tc.tile_pool(name="xtp", bufs=2) as xtp,
                    tc.tile_pool(name="rt", bufs=1) as rt,
                    tc.tile_pool(name="psB", bufs=2, space="PSUM") as psB,
                    tc.tile_pool(name="psC", bufs=2, space="PSUM") as psC,
                ):
                    logits_sb = rt.tile([NES, SEQ], f32)
                    iseq = rt.tile([NES, SEQ], f32)
                    # tsel overwrites logits in place (logits chunk is dead
                    # once gmax+iseq for that chunk are computed)
                    tsel = logits_sb
                    gmax = rt.tile([NES, G], f32)
                    tid_f = rt.tile([NES, G], f32)
                    xt_tiles = []
                    # DMA queue order: xt0 hi, xt0 lo, small consts, xb0,
                    # xt1 hi/lo, xb1, ...
                    for tc4 in range(SEQ // 512):
                        xt_t = xtp.tile([128, 2, KT, 512], bfl, tag="xt",
                                        name=f"xt_{tc4}")
                        nc.sync.dma_start(out=xt_t[:, 0], in_=xthl[tc4, :, 0])
                        nc.sync.dma_start(out=xt_t[:, 1], in_=xthl[tc4, :, 1])
                        xt_tiles.append(xt_t)
                        if tc4 == 0:
                            nc.sync.dma_start(out=ident_sb[:], in_=ident)
                            nc.sync.dma_start(out=glmask_sb[:], in_=glmask)
                            nc.sync.dma_start(out=tokid_sb[:], in_=tokid)
                        nc.sync.dma_start(
                            out=x_sb[:, 4 * tc4:4 * tc4 + 4, :],
                            in_=xbf[:, 4 * tc4:4 * tc4 + 4, :])
                        if tc4 == 3:
                            nc.sync.dma_start(out=bias_sb[:], in_=bias2)
                            prefetch_f1(0)
                            prefetch_f1(1)

                    def emit_logits(xt_t, psum_l, c0, c1, first, last):
                        # fp32-split logits, hi terms first so the PE can
                        # start before the lo half of the chunk lands
                        for k in range(KT):
                            nc.tensor.matmul(psum_l[:, c0:c1],
                                             ch_sb[:, k, :],
                                             xt_t[:, 0, k, c0:c1],
                                             start=(first and k == 0),
                                             stop=False)
                            nc.tensor.matmul(psum_l[:, c0:c1],
                                             cl_sb[:, k, :],
                                             xt_t[:, 0, k, c0:c1],
                                             start=False, stop=False)
                        for k in range(KT):
                            nc.tensor.matmul(psum_l[:, c0:c1],
                                             ch_sb[:, k, :],
                                             xt_t[:, 1, k, c0:c1],
                                             start=False,
                                             stop=(last and k == KT - 1))

                    def emit_routing(psum_l, t0, t1, g0, g1, c0, c1):
                        # tiebreak add during PSUM->SBUF, then group argmax
                        nc.vector.tensor_tensor(
                            out=logits_sb[:, t0:t1],
                            in0=psum_l[:, c0:c1], in1=tie_sb[:, c0:c1],
                            op=mybir.AluOpType.add)
                        ng = g1 - g0
                        lg = logits_sb[:, t0:t1].rearrange(
                            "e (g t) -> e g t", t=NE)
                        nc.vector.tensor_reduce(
                            out=gmax[:, g0:g1], in_=lg,
                            axis=mybir.AxisListType.X, op=mybir.AluOpType.max)
                        nc.vector.tensor_tensor(
                            out=iseq[:, t0:t1].rearrange(
                                "e (g t) -> e g t", t=NE),
                            in0=lg,
                            in1=gmax[:, g0:g1].unsqueeze(2).to_broadcast(
                                [NES, ng, NE]),
                            op=mybir.AluOpType.is_equal)
                        nc.gpsimd.tensor_tensor(
                            out=tsel[:, t0:t1], in0=iseq[:, t0:t1],
                            in1=tokid_sb[:, t0:t1], op=mybir.AluOpType.mult)
                        nc.vector.tensor_reduce(
                            out=tid_f[:, g0:g1],
                            in_=tsel[:, t0:t1].rearrange(
                                "e (g t) -> e g t", t=NE),
                            axis=mybir.AxisListType.X, op=mybir.AluOpType.max)

                    def emit_perm(tt):
                        # PE transpose + fused mask-broadcast mult on V
                        psum_t = psC.tile([128, NES], f32, space="PSUM")
                        nc.tensor.transpose(
                            out=psum_t[:],
                            in_=iseq[:, tt * 128:(tt + 1) * 128],
                            identity=ident_sb[:])
                        nc.vector.tensor_tensor(
                            out=perm_all[:, tt, :, :],
                            in0=psum_t.unsqueeze(2).to_broadcast(
                                [128, NES, GPT]),
                            in1=glmask_sb.unsqueeze(1).to_broadcast(
                                [128, NES, GPT]),
                            op=mybir.AluOpType.mult)

                    # chunk 0: fine-grained sub-chunk pipeline to minimize
                    # the cold-start routing-chain stall
                    xt_t = xt_tiles[0]
                    psum_l0 = psB.tile([NES, 512], f32, space="PSUM",
                                       tag="l", name="psl_0")
                    emit_logits(xt_t, psum_l0, 0, 256, True, True)
                    emit_routing(psum_l0, 0, 256, 0, 16, 0, 256)
                    emit_logits(xt_t, psum_l0, 256, 512, True, True)
                    emit_routing(psum_l0, 256, 512, 16, 32, 256, 512)
                    emit_perm(0)
                    emit_perm(1)
                    for i in range(0, 4):
                        emit_disp_chunk(psD, 0, i)
                    emit_perm(2)
                    emit_perm(3)

                    for tc4 in range(1, SEQ // 512):
                        xt_t = xt_tiles[tc4]
                        t0, t1 = tc4 * 512, (tc4 + 1) * 512
                        g0, g1 = tc4 * 32, (tc4 + 1) * 32
                        psum_l = psB.tile([NES, 512], f32, space="PSUM",
                                          tag="l", name=f"psl_{tc4}")
                        emit_logits(xt_t, psum_l, 0, 512, True, True)
                        emit_routing(psum_l, t0, t1, g0, g1, 0, 512)
                        # dispatch lagging one chunk behind routing (chunk 0
                        # already emitted its first half inline)
                        start = 4 if tc4 == 1 else 8 * (tc4 - 1)
                        for i in range(start, 8 * tc4):
                            emit_disp_chunk(psD, 0, i)
                        for tt in range(4 * tc4, 4 * tc4 + 4):
                            emit_perm(tt)
                    # trailing dispatch chunks
                    for i in range(24, 32):
                        emit_disp_chunk(psD, 0, i)
                    # export routing table for the host-side combine
                    tid_i32 = rt.tile([NES, G], i32)
                    nc.vector.tensor_copy(out=tid_i32[:], in_=tid_f[:])
                    nc.sync.dma_start(out=tids, in_=tid_i32[:])

                # ---- FFN phase ----
                with (
                    tc.tile_pool(name="dcp", bufs=1) as dcp,
                    tc.tile_pool(name="wp2", bufs=4) as wp2,
                    tc.tile_pool(name="yp", bufs=3) as yp,
                ):
                    prefetch_f1(2)
                    prefetch_f1(3)
                    for ac in range(3):
                        prefetch_f2(ac)
                    with tc.tile_pool(name="psH", bufs=2,
                                      space="PSUM") as psH:
                        for j in range(PPH):
                            emit_up_pair(psH, 0, j)
                    with tc.tile_pool(name="psY", bufs=1,
                                      space="PSUM") as psY:
                        for j in range(PPH):
                            # f1 octets for half 1: all up0 reads are done
                            # here, so these loads start immediately and
                            # never head-of-line-block the DMA queues
                            if j % 4 == 1:
                                prefetch_f1(4 + j // 4)
                            emit_down_pair(psY, 0, j)
                            for i in range(2 * j, 2 * j + 2):
                                emit_disp_chunk(psD, 1, i)
                    psD_cm.__exit__(None, None, None)
                    with tc.tile_pool(name="psH", bufs=4,
                                      space="PSUM") as psH:
                        for j in range(PPH):
                            emit_up_pair(psH, 1, j)
                    with tc.tile_pool(name="psY", bufs=2,
                                      space="PSUM") as psY:
                        for j in range(PPH):
                            emit_down_pair(psY, 1, j)

    nc.compile()
    return nc


def _host_prep(x, controller, f1, f2, bias):
    """Returns (shared_map, per_core_maps)."""
    x = np.asarray(x, dtype=np.float32)
    c2 = np.ascontiguousarray(np.asarray(controller, np.float32).reshape(DM, NES))
    c2h = c2.astype(bf16)
    c2l = (c2 - c2h.astype(np.float32)).astype(bf16)
    # pre-tile [dm, e] -> [p, k, e]
    c2h = np.ascontiguousarray(c2h.reshape(KT, 128, NES).transpose(1, 0, 2))
    c2l = np.ascontiguousarray(c2l.reshape(KT, 128, NES).transpose(1, 0, 2))
    f1m = np.asarray(f1, np.float32).reshape(DM, NES * ESZ).astype(bf16)
    # [octet, p, k, 512]: f1w[ag, p, k, q] = f1m[k*128+p, ag*512+q]
    f1w = np.ascontiguousarray(
        f1m.reshape(KT, 128, 8, 512).transpose(2, 1, 0, 3))
    # f2 stacked in es pairs: [(pair-parity f), pair, DM]
    f2p = np.asarray(f2, np.float32).reshape(NPAIR, 2, ESZ, DM)
    f2w = f2p.transpose(1, 2, 0, 3).reshape(128, NPAIR, DM)
    # pre-tile per 4-pair chunk: [ac, 128, 4, DM]
    f2w = np.ascontiguousarray(
        f2w.reshape(128, 8, 4, DM).transpose(1, 0, 2, 3)).astype(bf16)
    b2 = np.asarray(bias, np.float32).reshape(NPAIR, 2, ESZ)
    bias2 = np.ascontiguousarray(b2.transpose(1, 2, 0).reshape(128, NPAIR))
    tie = np.linspace(0.0, 1e-6, NE, dtype=np.float32)
    tie512 = np.broadcast_to(np.tile(tie, 512 // NE), (NES, 512)).copy()
    tokid = np.broadcast_to(np.arange(SEQ, dtype=np.float32), (NES, SEQ)).copy()
    ident = np.eye(NES, dtype=np.float32)
    gl = (np.arange(128) // NE)[:, None] == np.arange(GPT)[None, :]
    glmask = np.ascontiguousarray(gl.astype(bf16))
    shared = dict(c2h=c2h, c2l=c2l, f1w=f1w, f2w=f2w, bias2=bias2,
                  tie512=tie512, tokid=tokid, ident=ident, glmask=glmask)
    per_core = []
    for b in range(NCORES):
        xb = x[b]
        xT = np.ascontiguousarray(xb.T)
        xTh = xT.astype(bf16)
        xTl = (xT - xTh.astype(np.float32)).astype(bf16)
        # [chunk, p, hl, k, tok]: xthl[c, p, h, k, t] = xThl[k*128+p, c*512+t]
        xthl = np.stack([xTh, xTl], axis=1)          # [DM, 2, SEQ]
        xthl = np.ascontiguousarray(
            xthl.reshape(KT, 128, 2, 4, 512).transpose(3, 1, 2, 0, 4))
        per_core.append(dict(
            xthl=xthl,
            xbf=np.ascontiguousarray(
                xb.astype(bf16).reshape(NTT, 128, DM).transpose(1, 0, 2)),
        ))
    return shared, per_core


def _run(inputs, trace=False, tmpdir=None, trace_cores=None):
    if "nc" not in _CACHE:
        _CACHE["nc"] = _build_program()
    nc = _CACHE["nc"]
    shared, per_core = _host_prep(
        inputs["x"], inputs["controller"], inputs["f1"], inputs["f2"],
        inputs["bias"])
    in_maps = [dict(shared, **pc) for pc in per_core]
    res = run_bass_kernel_spmd(
        nc, in_maps, core_ids=list(range(NCORES)), trace=trace, tmpdir=tmpdir,
        trace_cores=trace_cores)
    out = np.zeros((NCORES, SEQ, DM), dtype=np.float32)
    for b in range(NCORES):
        # stage3[g, es, :] holds y for (es, group g); tids[es, g] is the token
        st = np.asarray(res.results[b]["stage3"]).astype(np.float32)
        tid = np.asarray(res.results[b]["tids"]).reshape(NES, G)
        np.add.at(out[b], tid.T.reshape(-1), st.reshape(G * NES, DM))
    return out, res


def kernel(**inputs) -> np.ndarray:
    out, _ = _run(inputs)
    return out
